# revision 7
# baseline (speedup 1.0000x reference)
"""Grouped GEMM (MoE expert-parallel) Trainium2 kernel.

Problem: inp [16384, 4096] f32, weight [8, 4096, 4096] f32 ([e, out_f, in_d]),
tokens pre-grouped by expert, 2048 tokens/expert.
out[e*2048+m, f] = sum_d inp[e*2048+m, d] * weight[e, f, d].

Strategy: expert-parallel, one expert per NeuronCore (8 cores), no
collectives. Matmuls run in bf16 (1 row/cycle like f32r, but half the SBUF
footprint and half the HBM traffic; measured fro rel err ~1.6e-3 vs the
2e-2 gate). The f32r baseline was DMA-coupled: 256MB of traffic/core
(weights read twice + a 64MB split-K partial round-trip) kept the DMA 96%
busy, starved the PE (84% busy) and HAM-throttled it 13% of the span. In
bf16 the whole activation block x_e^T [D, M] (16MB) stays resident in SBUF,
weights stream exactly once (32MB), no partial round-trip: 80MB total
traffic vs a 874us PE floor (2^21 moving rows @ 2.4GHz).

Host-side layout per core: xt = x_e^T [D, M] bf16 (d-major so the
contraction lands on SBUF partitions), wt = w_e^T regrouped to
[fo, p, ko, fm] (one contiguous 8KB/partition DMA per 128-wide f tile),
out ot = [F, M] f32, host transposes back while gathering.

Schedule: xt loads once as 64 [128, 1024] chunks (scalar-engine rings).
Warmup interleaves the first 4 f-tiles k-outer (8 PSUM banks) so the PE
chases the arriving xt chunks instead of stalling on the full 16MB load;
the remaining 28 f-tiles run k-inner over the full M with weight tiles
prefetched 2 ahead on the sync rings.
"""

import numpy as np

E = 8
M = 2048  # tokens per expert
D = 4096  # in features (contraction)
F = 4096  # out features
P = 128

KO = D // P  # 32 k-subtiles
FO = F // P  # 32 f tiles
MSEG = 512  # moving free dim per matmul (PSUM bank limit)
NSEG = M // MSEG  # 4
MH = 2
MB = M // MH  # 1024 (warmup half-blocks / evict granularity)
WARM_FO = 4  # f tiles interleaved k-outer during warmup

_cache = {}


def _build_nc():
    import concourse.mybir as mybir
    import concourse.tile as tile
    from concourse import bacc

    f32 = mybir.dt.float32
    bf16 = mybir.dt.bfloat16

    nc = bacc.Bacc(None, target_bir_lowering=False, debug=False)

    xt_d = nc.dram_tensor("xt", [D, M], bf16, kind="ExternalInput")
    wt_d = nc.dram_tensor("wt", [F, D], bf16, kind="ExternalInput")
    ot_d = nc.dram_tensor("ot", [F, M], f32, kind="ExternalOutput")

    xt_r = xt_d[:].rearrange("(ko p) m -> p ko m", p=P)  # [128, 32, 2048]
    # host pre-grouped: row fo*128+p, col ko*128+fm = weight[fo*128+fm, ko*128+p]
    wt_r = wt_d[:].rearrange("(fo p) x -> p fo x", p=P)  # [128, 32, 4096]
    ot_r = ot_d[:].rearrange("(fo p) m -> p fo m", p=P)  # [128, 32, 2048]

    with tile.TileContext(nc) as tc:
        with (
            tc.tile_pool(name="xres", bufs=1) as xres,
            tc.tile_pool(name="wstream", bufs=6) as wstream,
            tc.tile_pool(name="evict", bufs=4) as evict,
            tc.tile_pool(name="psum", bufs=8, space="PSUM") as psum,
        ):
            xt_sb = xres.tile([P, KO, M], bf16, tag="x")

            wtiles = {}

            def load_w(fo, chunks=1):
                # chunked loads for the first tiles so the PE's first
                # LDWEIGHTS doesn't wait on a full 1MB transfer
                w_sb = wstream.tile([P, KO * P], bf16, tag="w")
                cw = KO * P // chunks
                for c in range(chunks):
                    nc.sync.dma_start(
                        w_sb[:, c * cw : (c + 1) * cw],
                        wt_r[:, fo, c * cw : (c + 1) * cw],
                    )
                wtiles[fo] = w_sb

            # first warmup weight tiles ahead of the bulk xt stream
            load_w(0, chunks=8)
            load_w(1, chunks=4)
            for k in range(KO):
                if k < 4:
                    # halved first chunks: first matmuls start ~1us earlier
                    nc.scalar.dma_start(xt_sb[:, k, 0:MSEG], xt_r[:, k, 0:MSEG])
                    nc.scalar.dma_start(xt_sb[:, k, MSEG:MB], xt_r[:, k, MSEG:MB])
                else:
                    nc.scalar.dma_start(xt_sb[:, k, 0:MB], xt_r[:, k, 0:MB])
                if k == 2:
                    load_w(2, chunks=2)
                if k == 5:
                    load_w(3, chunks=2)
            for k in range(KO):
                nc.scalar.dma_start(xt_sb[:, k, MB:M], xt_r[:, k, MB:M])

            def evict_ps(fo, mh, ps_pair):
                ot_sb = evict.tile([P, MB], f32, tag="ev")
                for s in range(2):
                    nc.vector.tensor_copy(
                        ot_sb[:, s * MSEG : (s + 1) * MSEG], ps_pair[s]
                    )
                nc.sync.dma_start(ot_r[:, fo, mh * MB : (mh + 1) * MB], ot_sb[:])

            # warmup: f tiles 0..3 k-outer per m-half, chasing the xt chunks
            for mh in range(MH):
                ps = {
                    (fo, s): psum.tile(
                        [P, MSEG], f32, tag="acc", name=f"ps_w{mh}_{fo}_{s}"
                    )
                    for fo in range(WARM_FO)
                    for s in range(2)
                }
                for k in range(KO):
                    for fo in range(WARM_FO):
                        w_sb = wtiles[fo]
                        for s in range(2):
                            m0 = mh * MB + s * MSEG
                            nc.tensor.matmul(
                                ps[fo, s],
                                w_sb[:, k * P : (k + 1) * P],
                                xt_sb[:, k, m0 : m0 + MSEG],
                                start=(k == 0),
                                stop=(k == KO - 1),
                            )
                if mh == 0:
                    load_w(WARM_FO)
                    load_w(WARM_FO + 1)
                for fo in range(WARM_FO):
                    evict_ps(fo, mh, [ps[fo, 0], ps[fo, 1]])

            # main loop: k-inner over full M, weights prefetched 2 ahead
            for fo in range(WARM_FO, FO - 1):
                if fo + 2 < FO:
                    load_w(fo + 2)
                w_sb = wtiles.pop(fo)
                ps = [
                    psum.tile([P, MSEG], f32, tag="acc", name=f"ps_{fo}_{s}")
                    for s in range(NSEG)
                ]
                for k in range(KO):
                    for s in range(NSEG):
                        nc.tensor.matmul(
                            ps[s],
                            w_sb[:, k * P : (k + 1) * P],
                            xt_sb[:, k, s * MSEG : (s + 1) * MSEG],
                            start=(k == 0),
                            stop=(k == KO - 1),
                        )
                for mh in range(MH):
                    evict_ps(fo, mh, ps[mh * 2 : mh * 2 + 2])

            # last f tile: s-outer so each m-segment closes early and its
            # eviction + out-DMA overlap the remaining matmuls
            fo = FO - 1
            w_sb = wtiles.pop(fo)
            for s in range(NSEG):
                ps_s = psum.tile([P, MSEG], f32, tag="acc", name=f"ps_last_{s}")
                for k in range(KO):
                    nc.tensor.matmul(
                        ps_s,
                        w_sb[:, k * P : (k + 1) * P],
                        xt_sb[:, k, s * MSEG : (s + 1) * MSEG],
                        start=(k == 0),
                        stop=(k == KO - 1),
                    )
                ot_sb = evict.tile([P, MSEG], f32, tag="ev", name=f"ev_last_{s}")
                nc.vector.tensor_copy(ot_sb[:], ps_s)
                nc.sync.dma_start(
                    ot_r[:, fo, s * MSEG : (s + 1) * MSEG], ot_sb[:]
                )

    nc.compile()
    return nc


def _get_nc():
    if "nc" not in _cache:
        _cache["nc"] = _build_nc()
    return _cache["nc"]


def make_in_maps(inp, weight):
    import ml_dtypes

    bf16 = ml_dtypes.bfloat16
    xb = np.asarray(inp).astype(bf16)
    wb = np.asarray(weight).astype(bf16)
    in_maps = []
    for e in range(E):
        xt = np.ascontiguousarray(xb[e * M : (e + 1) * M].T)  # [D, M]
        # [fo, p, ko, fm] flattened to [F, D]
        wt = np.ascontiguousarray(
            wb[e].reshape(FO, P, KO, P).transpose(0, 3, 2, 1)
        ).reshape(F, D)
        in_maps.append({"xt": xt, "wt": wt})
    return in_maps


def gather_out(res):
    out = np.empty((E * M, F), dtype=np.float32)
    for e in range(E):
        out[e * M : (e + 1) * M] = res.results[e]["ot"].T
    return out


def kernel(inp, weight, num_tokens_per_expert):
    from concourse.bass_utils import run_bass_kernel_spmd

    inp = np.asarray(inp)
    weight = np.asarray(weight)
    assert inp.shape == (E * M, D) and weight.shape == (E, F, D)

    nc = _get_nc()
    res = run_bass_kernel_spmd(nc, make_in_maps(inp, weight), list(range(E)))
    return gather_out(res)
